# revision 8
# baseline (speedup 1.0000x reference)
"""Self-contained 8-core Trainium2 Bass kernel for the 6-layer transformer
encoder (B=4, S=1024, D=768, H=12, F=3072).

Sharding: each core owns (batch b = c//2, sequence half c%2) = 512 tokens.
All weights replicated. Per layer, K and V for the partner's sequence half
are exchanged via 8-way AllGather + indirect-DMA row selection (offsets are
host-provided per core, so the SPMD graph stays core-independent).

Layout: activations are kept feature-major ("T" suffix: [feat, tok]) so
LayerNorm stats use ones-matmul partition reductions and all linear layers
are plain accumulating matmuls. V is produced token-major directly by
swapping the matmul operand roles. Softmax denominators ride along as a
65th ones-column in the V stationary operand. All matmuls run in float32r
(1 cycle/row at N=512, ~13-bit mantissa).
"""
import numpy as np

import concourse.bass as bass
import concourse.tile as tile
from concourse import bacc, mybir, bass_utils

F32 = mybir.dt.float32
F32R = mybir.dt.float32r
I32 = mybir.dt.int32
ACTF = mybir.ActivationFunctionType
ALU = mybir.AluOpType

NCORES = 8
T = 512          # tokens per core
D = 768          # model dim
KD = D // 128    # 6 feature chunks
NH = 12          # heads
HD = 64          # head dim
FF = 3072        # ffn hidden
FT = FF // 128   # 24
L = 6
EPS = 1e-5
INV_D = 1.0 / D
SCALE = 0.125    # 1/sqrt(64)

VROW = NH * (HD + 1)   # 780: V_aug row width (ones col per head)


def build_bass(n_layers=L, final_ln=True, taps=False):
    nc = bacc.Bacc("TRN2", target_bir_lowering=False, debug=False,
                   num_devices=NCORES)
    d = {}
    def din(name, shape, dt=F32):
        d[name] = nc.dram_tensor(name, list(shape), dt, kind="ExternalInput").ap()
    din("xT", [D, T])
    din("wqk", [L, 12, 128, D])
    din("wv", [L, KD, 128, D])
    din("bqk", [L, 128, 12])
    din("bvrow", [L, 1, D])
    din("wo_r", [L, KD, 128, D])
    din("bo_c", [L, 128, KD])
    din("w1_r", [L, FT, 128, D])
    din("b1_c", [L, 128, FT])
    din("w2_r", [L, FT, 128, D])
    din("b2_c", [L, 128, KD])
    din("g1n_c", [L, 128, KD])
    din("b1l_c", [L, 128, KD])
    din("g2n_c", [L, 128, KD])
    din("b2l_c", [L, 128, KD])
    din("gfn_c", [128, KD])
    din("bfl_c", [128, KD])
    din("cones", [128, 128])
    din("roff", [128, 10], I32)
    out = nc.dram_tensor("out", [D, T], F32, kind="ExternalOutput").ap()
    tap = {}
    if taps:
        for nm, shp in [("t_kT", [D, T]), ("t_qT", [D, T]), ("t_va0", [128, VROW]),
                        ("t_kTr", [D, T]), ("t_var0", [128, VROW]),
                        ("t_attnT", [D, T]), ("t_x1T", [D, T]), ("t_x1nT", [D, T]),
                        ("t_E00", [128, T]), ("t_pav00", [128, T])]:
            tap[nm] = nc.dram_tensor(nm, shp, F32, kind="ExternalOutput").ap()

    from contextlib import ExitStack
    with tile.TileContext(nc) as tc, ExitStack() as ctx:
        sbP = ctx.enter_context(tc.tile_pool(name="sbP", bufs=1))
        sbW = ctx.enter_context(tc.tile_pool(name="sbW", bufs=3))
        sbE = ctx.enter_context(tc.tile_pool(name="sbE", bufs=5))
        sbA = ctx.enter_context(tc.tile_pool(name="sbA", bufs=2))
        sbS = ctx.enter_context(tc.tile_pool(name="sbS", bufs=2))
        psA = ctx.enter_context(tc.tile_pool(name="psA", bufs=1, space="PSUM"))
        psW = ctx.enter_context(tc.tile_pool(name="psW", bufs=2, space="PSUM"))
        dram = ctx.enter_context(tc.tile_pool(name="dram", bufs=2, space="DRAM"))

        ones128 = sbP.tile([128, 128], F32R, tag="ones", name="ones128")
        nc.sync.dma_start(out=ones128, in_=d["cones"].bitcast(F32R))
        toff = sbP.tile([128, 10], I32, tag="toff", name="toff")
        nc.sync.dma_start(out=toff, in_=d["roff"])
        epsT = sbP.tile([128, 1], F32, tag="epsT", name="epsT")
        nc.vector.memset(epsT, EPS)

        def ptile(tag_prefix, i, shape=(128, T), dt=F32R):
            return sbP.tile(list(shape), dt, tag=f"{tag_prefix}{i}",
                            name=f"{tag_prefix}{i}")

        # initial hidden state (feature-major)
        hT = []
        for i in range(KD):
            t = ptile("hT", i)
            nc.sync.dma_start(out=t, in_=d["xT"][i * 128:(i + 1) * 128, :].bitcast(F32R))
            hT.append(t)

        def layernorm(src, dst_tag, gneg, bln, out_dt=F32R, acc_tags=("acc2", "acc3")):
            """dst = Identity(((mu - x) * rstd) * gneg + bln); returns dst tiles."""
            SB = psA.tile([128, T], F32, tag=acc_tags[0], name=f"SB_{dst_tag}")
            SQ = psA.tile([128, T], F32, tag=acc_tags[1], name=f"SQ_{dst_tag}")
            for kc in range(KD):
                nc.tensor.matmul(SB[:], ones128[:], src[kc][:],
                                 start=(kc == 0), stop=(kc == KD - 1))
            for kc in range(KD):
                sq = sbA.tile([128, T], F32R, tag="sq", name=f"sq_{dst_tag}{kc}")
                nc.scalar.activation(out=sq, in_=src[kc], func=ACTF.Square)
                nc.tensor.matmul(SQ[:], ones128[:], sq[:],
                                 start=(kc == 0), stop=(kc == KD - 1))
            m2 = sbS.tile([128, T], F32, tag="lnt", name=f"m2_{dst_tag}")
            nc.scalar.activation(out=m2, in_=SB, func=ACTF.Square, scale=INV_D)
            var = sbS.tile([128, T], F32, tag="lnt", name=f"var_{dst_tag}")
            nc.vector.scalar_tensor_tensor(out=var, in0=SQ, scalar=INV_D, in1=m2,
                                           op0=ALU.mult, op1=ALU.subtract)
            sd = sbS.tile([128, T], F32, tag="lnt", name=f"sd_{dst_tag}")
            nc.scalar.activation(out=sd, in_=var, func=ACTF.Sqrt, bias=epsT[:, 0:1])
            rstd = sbS.tile([128, T], F32, tag="lnt2", name=f"rstd_{dst_tag}")
            nc.vector.reciprocal(out=rstd, in_=sd)
            dst = []
            for kc in range(KD):
                dd = sbS.tile([128, T], F32, tag="lnt", name=f"d_{dst_tag}{kc}")
                nc.vector.scalar_tensor_tensor(out=dd, in0=SB, scalar=INV_D,
                                               in1=src[kc], op0=ALU.mult,
                                               op1=ALU.subtract)
                ee = sbS.tile([128, T], F32, tag="lnt", name=f"e_{dst_tag}{kc}")
                nc.vector.tensor_mul(out=ee, in0=dd, in1=rstd)
                o = ptile(dst_tag, kc, dt=out_dt)
                nc.scalar.activation(out=o, in_=ee, func=ACTF.Identity,
                                     scale=gneg[:, kc:kc + 1], bias=bln[:, kc:kc + 1])
                dst.append(o)
            return dst

        for l in range(n_layers):
            # per-layer bias/gain tiles
            bqk_t = sbP.tile([128, 12], F32, tag="bqk", name=f"bqk{l}")
            nc.sync.dma_start(out=bqk_t, in_=d["bqk"][l])
            bo_t = sbP.tile([128, KD], F32, tag="bo", name=f"bo{l}")
            nc.sync.dma_start(out=bo_t, in_=d["bo_c"][l])
            b1_t = sbP.tile([128, FT], F32, tag="b1", name=f"b1{l}")
            nc.sync.dma_start(out=b1_t, in_=d["b1_c"][l])
            b2_t = sbP.tile([128, KD], F32, tag="b2", name=f"b2{l}")
            nc.sync.dma_start(out=b2_t, in_=d["b2_c"][l])
            g1n_t = sbP.tile([128, KD], F32, tag="g1n", name=f"g1n{l}")
            nc.sync.dma_start(out=g1n_t, in_=d["g1n_c"][l])
            b1l_t = sbP.tile([128, KD], F32, tag="b1l", name=f"b1l{l}")
            nc.sync.dma_start(out=b1l_t, in_=d["b1l_c"][l])
            g2n_t = sbP.tile([128, KD], F32, tag="g2n", name=f"g2n{l}")
            nc.sync.dma_start(out=g2n_t, in_=d["g2n_c"][l])
            b2l_t = sbP.tile([128, KD], F32, tag="b2l", name=f"b2l{l}")
            nc.sync.dma_start(out=b2l_t, in_=d["b2l_c"][l])
            bvr = sbS.tile([1, D], F32, tag="bvr", name=f"bvr{l}")
            nc.sync.dma_start(out=bvr, in_=d["bvrow"][l])
            bvb = sbP.tile([128, D], F32, tag="bvb", name=f"bvb{l}")
            nc.gpsimd.partition_broadcast(bvb[:], bvr[0:1, :])

            # ---- Phase A: K projection ----
            kT = []
            for ot in range(KD):
                wt = sbW.tile([128, D], F32R, tag="wtile", name=f"wk{l}_{ot}")
                nc.sync.dma_start(out=wt, in_=d["wqk"][l, 6 + ot].bitcast(F32R))
                pk = psW.tile([128, T], F32, tag="w", name=f"pk{l}_{ot}")
                for kc in range(KD):
                    nc.tensor.matmul(pk[:], wt[:, kc * 128:(kc + 1) * 128],
                                     hT[kc][:], start=(kc == 0), stop=(kc == KD - 1))
                t = ptile("kT", ot)
                nc.scalar.activation(out=t, in_=pk, func=ACTF.Identity,
                                     bias=bqk_t[:, 6 + ot:7 + ot])
                kT.append(t)

            # K bounce + AllGather
            agk_in = dram.tile([D, T], F32, tag="agk_in", name=f"agk_in{l}")
            agk_out = dram.tile([NCORES * D, T], F32, tag="agk_out",
                                name=f"agk_out{l}", addr_space="Shared")
            for i in range(KD):
                nc.sync.dma_start(out=agk_in[i * 128:(i + 1) * 128, :],
                                  in_=kT[i].bitcast(F32))
            nc.gpsimd.collective_compute(
                "AllGather", ALU.bypass, ins=[agk_in.opt()], outs=[agk_out.opt()],
                replica_groups=[list(range(NCORES))])
            kTr = []
            for i in range(KD):
                t = ptile("kTr", i)
                nc.gpsimd.indirect_dma_start(
                    out=t[:], out_offset=None, in_=agk_out.bitcast(F32R)[:],
                    in_offset=bass.IndirectOffsetOnAxis(ap=toff[:, i:i + 1], axis=0))
                kTr.append(t)

            # ---- Phase A2: V projection (token-major, with ones cols) ----
            vslab = []
            for kc in range(KD):
                w = sbP.tile([128, D], F32R, tag=f"vslab{kc}", name=f"wv{l}_{kc}")
                nc.sync.dma_start(out=w, in_=d["wv"][l, kc].bitcast(F32R))
                vslab.append(w)
            va = []
            for tt in range(4):
                t = sbP.tile([128, NH, HD + 1], F32R, tag=f"va{tt}", name=f"va{l}_{tt}")
                # ones columns (slot 64 of each head)
                nc.sync.dma_start(out=t[:, :, HD:HD + 1],
                                  in_=d["cones"][:, 0:NH].bitcast(F32R))
                va.append(t)
            for ng in range(2):
                ncols = 512 if ng == 0 else 256
                h0 = 8 if ng == 0 else 4
                for tt in range(4):
                    pv = psW.tile([128, T], F32, tag="w", name=f"pv{l}_{ng}_{tt}")
                    for kc in range(KD):
                        nc.tensor.matmul(
                            pv[:, 0:ncols],
                            hT[kc][:, tt * 128:(tt + 1) * 128],
                            vslab[kc][:, ng * 512:ng * 512 + ncols],
                            start=(kc == 0), stop=(kc == KD - 1))
                    dst = va[tt][:, (0 if ng == 0 else 8):(8 if ng == 0 else 12), 0:HD]
                    nc.vector.tensor_tensor(
                        out=dst,
                        in0=pv[:, 0:ncols].rearrange("p (h c) -> p h c", c=HD),
                        in1=bvb[:, ng * 512:ng * 512 + ncols].rearrange(
                            "p (h c) -> p h c", c=HD),
                        op=ALU.add)

            # V bounce + AllGather
            agv_in = dram.tile([T, VROW], F32, tag="agv_in", name=f"agv_in{l}")
            agv_out = dram.tile([NCORES * T, VROW], F32, tag="agv_out",
                                name=f"agv_out{l}", addr_space="Shared")
            agv_in_v = agv_in.rearrange("(tt p) v -> tt p v", p=128)
            for tt in range(4):
                nc.sync.dma_start(
                    out=agv_in_v[tt],
                    in_=va[tt].rearrange("p h c -> p (h c)").bitcast(F32))
            nc.gpsimd.collective_compute(
                "AllGather", ALU.bypass, ins=[agv_in.opt()], outs=[agv_out.opt()],
                replica_groups=[list(range(NCORES))])
            var_ = []
            for j in range(4):
                t = sbP.tile([128, NH, HD + 1], F32R, tag=f"var{j}", name=f"var{l}_{j}")
                nc.gpsimd.indirect_dma_start(
                    out=t.rearrange("p h c -> p (h c)")[:], out_offset=None,
                    in_=agv_out.bitcast(F32R)[:],
                    in_offset=bass.IndirectOffsetOnAxis(ap=toff[:, 6 + j:7 + j], axis=0))
                var_.append(t)

            # ---- Phase A3: Q projection ----
            qT = []
            for ot in range(KD):
                wt = sbW.tile([128, D], F32R, tag="wtile", name=f"wq{l}_{ot}")
                nc.sync.dma_start(out=wt, in_=d["wqk"][l, ot].bitcast(F32R))
                pq = psW.tile([128, T], F32, tag="w", name=f"pq{l}_{ot}")
                for kc in range(KD):
                    nc.tensor.matmul(pq[:], wt[:, kc * 128:(kc + 1) * 128],
                                     hT[kc][:], start=(kc == 0), stop=(kc == KD - 1))
                t = ptile("qT", ot)
                nc.scalar.activation(out=t, in_=pq, func=ACTF.Identity,
                                     bias=bqk_t[:, ot:ot + 1])
                qT.append(t)

            if taps and l == 0:
                for i in range(KD):
                    nc.sync.dma_start(out=tap["t_kT"][i*128:(i+1)*128, :], in_=kT[i].bitcast(F32))
                    nc.sync.dma_start(out=tap["t_qT"][i*128:(i+1)*128, :], in_=qT[i].bitcast(F32))
                    nc.sync.dma_start(out=tap["t_kTr"][i*128:(i+1)*128, :], in_=kTr[i].bitcast(F32))
                nc.sync.dma_start(out=tap["t_va0"], in_=va[0].rearrange("p h c -> p (h c)").bitcast(F32))
                nc.sync.dma_start(out=tap["t_var0"], in_=var_[0].rearrange("p h c -> p (h c)").bitcast(F32))

            # ---- Phase B: attention (per head pair) ----
            attnT = [ptile("attnT", i) for i in range(KD)]
            for hp in range(KD):
                pav = [psA.tile([128, T], F32, tag=f"acc{sl}", name=f"pav{l}_{hp}_{sl}")
                       for sl in range(2)]
                for kth in range(8):
                    ksrc = kT[hp] if kth < 4 else kTr[hp]
                    ko = (kth % 4) * 128
                    vsrc = va[kth % 4] if kth < 4 else var_[kth % 4]
                    for sl in range(2):
                        h = 2 * hp + sl
                        pe = psW.tile([128, T], F32, tag="w", name=f"pe{l}_{hp}_{kth}_{sl}")
                        nc.tensor.matmul(pe[:],
                                         ksrc[sl * 64:sl * 64 + 64, ko:ko + 128],
                                         qT[hp][sl * 64:sl * 64 + 64, :],
                                         start=True, stop=True)
                        E = sbE.tile([128, T], F32R, tag="E", name=f"E{l}_{hp}_{kth}_{sl}")
                        nc.scalar.activation(out=E, in_=pe, func=ACTF.Exp, scale=SCALE)
                        nc.tensor.matmul(pav[sl][0:65, :], vsrc[:, h, :], E[:],
                                         start=(kth == 0), stop=(kth == 7))
                        if taps and l == 0 and hp == 0 and kth == 0 and sl == 0:
                            nc.sync.dma_start(out=tap["t_E00"], in_=E.bitcast(F32))
                if taps and l == 0 and hp == 0:
                    pav_sb = sbS.tile([128, T], F32, tag="pavsb", name="pav_sb")
                    nc.vector.tensor_copy(out=pav_sb[0:65, :], in_=pav[0][0:65, :])
                    nc.sync.dma_start(out=tap["t_pav00"][0:65, :], in_=pav_sb[0:65, :])
                for sl in range(2):
                    srow = sbS.tile([1, T], F32, tag="srow", name=f"srow{l}_{hp}_{sl}")
                    nc.vector.tensor_copy(out=srow[0:1, :], in_=pav[sl][64:65, :])
                    rec = sbS.tile([1, T], F32, tag="rec", name=f"rec{l}_{hp}_{sl}")
                    nc.vector.reciprocal(out=rec, in_=srow)
                    rb = sbS.tile([64, T], F32, tag="rb", name=f"rb{l}_{hp}_{sl}")
                    nc.gpsimd.partition_broadcast(rb[:], rec[0:1, :], channels=64)
                    nc.vector.tensor_mul(out=attnT[hp][sl * 64:sl * 64 + 64, :],
                                         in0=pav[sl][0:64, :], in1=rb[0:64, :])

            # ---- Phase C: out-projection + residual + LN1 ----
            x1T = []
            for ot in range(KD):
                wt = sbW.tile([128, D], F32R, tag="wtile", name=f"wo{l}_{ot}")
                nc.sync.dma_start(out=wt, in_=d["wo_r"][l, ot].bitcast(F32R))
                po = psW.tile([128, T], F32, tag="w", name=f"po{l}_{ot}")
                for kc in range(KD):
                    nc.tensor.matmul(po[:], wt[:, kc * 128:(kc + 1) * 128],
                                     attnT[kc][:], start=(kc == 0), stop=(kc == KD - 1))
                t = ptile("x1T", ot)
                nc.vector.scalar_tensor_tensor(out=t, in0=po,
                                               scalar=bo_t[:, ot:ot + 1],
                                               in1=hT[ot], op0=ALU.add, op1=ALU.add)
                x1T.append(t)
            if taps and l == 0:
                for i in range(KD):
                    nc.sync.dma_start(out=tap["t_attnT"][i*128:(i+1)*128, :], in_=attnT[i].bitcast(F32))
                    nc.sync.dma_start(out=tap["t_x1T"][i*128:(i+1)*128, :], in_=x1T[i].bitcast(F32))
            x1nT = layernorm(x1T, "x1nT", g1n_t, b1l_t)
            if taps and l == 0:
                for i in range(KD):
                    nc.sync.dma_start(out=tap["t_x1nT"][i*128:(i+1)*128, :], in_=x1nT[i].bitcast(F32))

            # ---- Phase D: FFN (fc1 + fc2 interleaved) + residual + LN2 ----
            pd = [psA.tile([128, T], F32, tag=f"acc{dt}", name=f"pd{l}_{dt}")
                  for dt in range(KD)]
            for ft in range(FT):
                w1t = sbW.tile([128, D], F32R, tag="wtile", name=f"w1{l}_{ft}")
                nc.sync.dma_start(out=w1t, in_=d["w1_r"][l, ft].bitcast(F32R))
                pf = psW.tile([128, T], F32, tag="w", name=f"pf{l}_{ft}")
                for kc in range(KD):
                    nc.tensor.matmul(pf[:], w1t[:, kc * 128:(kc + 1) * 128],
                                     x1nT[kc][:], start=(kc == 0), stop=(kc == KD - 1))
                aT = sbA.tile([128, T], F32R, tag="aT", name=f"aT{l}_{ft}")
                nc.scalar.activation(out=aT, in_=pf, func=ACTF.Relu,
                                     bias=b1_t[:, ft:ft + 1])
                w2t = sbW.tile([128, D], F32R, tag="w2tile", name=f"w2{l}_{ft}")
                nc.sync.dma_start(out=w2t, in_=d["w2_r"][l, ft].bitcast(F32R))
                for dt in range(KD):
                    nc.tensor.matmul(pd[dt][:], w2t[:, dt * 128:(dt + 1) * 128],
                                     aT[:], start=(ft == 0), stop=(ft == FT - 1))
            x2T = []
            for dt in range(KD):
                t = ptile("qT", dt)  # reuse qT slots (dead after attention)
                nc.vector.scalar_tensor_tensor(out=t, in0=pd[dt],
                                               scalar=b2_t[:, dt:dt + 1],
                                               in1=x1nT[dt], op0=ALU.add, op1=ALU.add)
                x2T.append(t)
            hT = layernorm(x2T, "hT", g2n_t, b2l_t)

        if final_ln:
            gfn_t = sbP.tile([128, KD], F32, tag="gfn", name="gfn")
            nc.sync.dma_start(out=gfn_t, in_=d["gfn_c"])
            bfl_t = sbP.tile([128, KD], F32, tag="bfl", name="bfl")
            nc.sync.dma_start(out=bfl_t, in_=d["bfl_c"])
            oT = layernorm(hT, "oT", gfn_t, bfl_t, out_dt=F32)
        else:
            oT = hT
        for i in range(KD):
            nc.sync.dma_start(out=out[i * 128:(i + 1) * 128, :],
                              in_=oT[i].bitcast(F32))
    nc.compile()
    return nc


def _pos_encoding(S, Dm):
    pos = np.arange(S, dtype=np.float32)[:, None]
    div = np.exp(np.arange(0, Dm, 2, dtype=np.float32) * (-np.log(10000.0) / Dm))
    pe = np.zeros((S, Dm), dtype=np.float32)
    pe[:, 0::2] = np.sin(pos * div)
    pe[:, 1::2] = np.cos(pos * div)
    return pe


def prep_inputs(x, Wqkv, bqkv, Wo, bo, ln1_g, ln1_b, W1, b1, W2, b2,
                ln2_g, ln2_b, lnf_g, lnf_b, num_heads):
    """Build the 8 per-core in_maps (host-side shard + re-layout)."""
    x = np.asarray(x, dtype=np.float32)
    B, S, Dm = x.shape
    pe = _pos_encoding(S, Dm)
    h0 = x + pe[None]

    Wqkv = np.ascontiguousarray(np.asarray(Wqkv, np.float32))
    bqkv = np.asarray(bqkv, np.float32)
    Wo = np.asarray(Wo, np.float32)
    W1 = np.asarray(W1, np.float32)
    W2 = np.asarray(W2, np.float32)

    def blocks(W, n_in, n_out):
        # [L, n_in*128, n_out*128] -> [L, n_out, 128(p=in), n_in*128(free=(kc j))]
        Lx = W.shape[0]
        r = W.reshape(Lx, n_in, 128, n_out, 128)
        return np.ascontiguousarray(r.transpose(0, 3, 2, 1, 4).reshape(
            Lx, n_out, 128, n_in * 128))

    wqk = blocks(Wqkv[:, :, :2 * D], KD, 12)          # q: ot 0..5, k: 6..11
    wv = np.ascontiguousarray(
        Wqkv[:, :, 2 * D:].reshape(L, KD, 128, D))     # natural slabs
    wo_r = blocks(Wo, KD, KD)
    w1_r = blocks(W1, KD, FT)
    w2_r = np.ascontiguousarray(W2.reshape(L, FT, 128, D))

    def cols(v, n):  # [L, n*128] -> [L, 128, n]
        return np.ascontiguousarray(
            np.asarray(v, np.float32).reshape(-1, n, 128).transpose(0, 2, 1))

    bqk_c = cols(bqkv[:, :2 * D], 12)
    bvrow = np.ascontiguousarray(bqkv[:, 2 * D:]).reshape(L, 1, D)
    bo_c = cols(np.asarray(bo, np.float32), KD)
    b1_c = cols(np.asarray(b1, np.float32), FT)
    b2_c = cols(np.asarray(b2, np.float32), KD)
    g1n_c = cols(-np.asarray(ln1_g, np.float32), KD)
    b1l_c = cols(np.asarray(ln1_b, np.float32), KD)
    g2n_c = cols(-np.asarray(ln2_g, np.float32), KD)
    b2l_c = cols(np.asarray(ln2_b, np.float32), KD)
    gfn_c = cols(-np.asarray(lnf_g, np.float32)[None], KD)[0]
    bfl_c = cols(np.asarray(lnf_b, np.float32)[None], KD)[0]
    cones = np.ones((128, 128), dtype=np.float32)

    shared = dict(wqk=wqk, wv=wv, bqk=bqk_c, bvrow=bvrow, wo_r=wo_r, bo_c=bo_c,
                  w1_r=w1_r, b1_c=b1_c, w2_r=w2_r, b2_c=b2_c, g1n_c=g1n_c,
                  b1l_c=b1l_c, g2n_c=g2n_c, b2l_c=b2l_c, gfn_c=gfn_c,
                  bfl_c=bfl_c, cones=cones)

    in_maps = []
    p = np.arange(128, dtype=np.int32)[:, None]
    for c in range(NCORES):
        b, half = c // 2, c % 2
        shard = h0[b, half * T:(half + 1) * T, :]        # [512, 768]
        xT = np.ascontiguousarray(shard.T)               # [768, 512]
        partner = c ^ 1
        roff = np.zeros((128, 10), dtype=np.int32)
        for j in range(KD):
            roff[:, j:j + 1] = partner * D + j * 128 + p
        for j in range(4):
            roff[:, 6 + j:7 + j] = partner * T + j * 128 + p
        in_maps.append({**shared, "xT": xT, "roff": roff})
    return in_maps


_CACHED_NC = None


def kernel(**inputs) -> np.ndarray:
    global _CACHED_NC
    in_maps = prep_inputs(**inputs)
    if _CACHED_NC is None:
        _CACHED_NC = build_bass()
    res = bass_utils.run_bass_kernel_spmd(
        _CACHED_NC, in_maps, core_ids=list(range(NCORES)))
    x = np.asarray(inputs["x"])
    B, S, Dm = x.shape
    out = np.empty((B, S, Dm), dtype=np.float32)
    for c in range(NCORES):
        b, half = c // 2, c % 2
        out[b, half * T:(half + 1) * T, :] = res.results[c]["out"].T
    return out
